# revision 23
# baseline (speedup 1.0000x reference)
"""Trainium2 Bass kernel for causal self-attention (nn_CausalSelfAttention).

Sharding: 8 cores = 4 batches x 2 head-groups (TP over heads).
Core c handles batch b=c//2, head-group g=c%2 (8 of 16 heads).
QKV column-parallel, c_proj row-parallel. No on-device collective: each
core writes its row-parallel partial of c_proj to DRAM and the host adds
the two partials of each pair (the unshard step).

Per-core device program (software-pipelined across phases):
  block i (512 query rows):
    attention(i) woven with qkv(i+1) and c_proj(i-1):
      - S^T tiles per head-PAIR into one [128, 2x512] PSUM region,
        ONE exp (ACT, bf16 out) per j-step, diagonal-trimmed widths,
        single [128,128] triangular mask applied multiplicatively on
        the 128-wide diagonal micro-block only (DVE).
      - PV accumulate per head into [65, 512] PSUM (row 64 = prob sums).
      - qkv(i+1): fp32r matmuls from resident weights; RMS-norm via
        ACT Square (same act table as Exp -> zero table reloads) +
        DVE reduce + DVE fast-inverse-sqrt (bit trick + 2 Newton
        iterations); zero-padded rotary on DVE batched over q&k;
        PE transposes to d-major with Pool-engine PSUM->SBUF copies.
      - c_proj(i-1) partial matmuls spliced after each pair's
        normalize (frees the y PSUM banks they reuse), Pool copy,
        DMA straight to the external output.
Engine balance: ACT = exp + square only (one act table, loaded once);
Pool = all PSUM->SBUF copies; DVE = reductions/rotary/masks/rsqrt.
PSUM: s-pool 4 banks, y/cproj-pool 2 banks, qkv/transpose-pool 2 banks.

Timing mode (loop_reps > 1) wraps the whole pipeline in a hardware
For_i; the program is identical otherwise (no collectives to skip).
"""

import numpy as np

import concourse.bass as bass
import concourse.mybir as mybir
import concourse.tile as tile
from concourse import bacc
from concourse.alu_op_type import AluOpType as OP
from concourse.bass_utils import run_bass_kernel_spmd
from concourse.masks import make_identity

F32 = mybir.dt.float32
F32R = mybir.dt.float32r
BF16 = mybir.dt.bfloat16
I32 = mybir.dt.int32
AFT = mybir.ActivationFunctionType
AX = mybir.AxisListType

ATTN_SCALE = 0.12
EPS = 1e-6
D = 64  # head dim
NFREQ = 16  # live rotary frequencies (rest of the 32 are identity)
NB_W = 512  # t-block width
N_CORES = 8
MAGIC = 0x5F3759DF


def build_nc(T, C, O, n_cores=N_CORES, loop_reps=1):
    """O = per-core output columns for each of q/k/v (= heads_per_core * 64)."""
    HLOC = O // D          # 8 local heads
    NPAIR = HLOC // 2
    CT = C // 128          # 8 contraction tiles for qkv
    TT = T // 128          # 16 t-tiles
    NB = T // NB_W         # 4 blocks
    SUB = NB_W // 128      # 4 sub-tiles per block
    ODT = O // 128         # 4 d-major tiles per projection
    OBW = min(512, C)
    OB = C // OBW

    nc = bacc.Bacc("TRN2", target_bir_lowering=False, debug=False,
                   num_devices=n_cores)

    xT = nc.dram_tensor("xT", [C, T], F32, kind="ExternalInput")
    wT = nc.dram_tensor("wT", [C, 3 * O], F32, kind="ExternalInput")
    cpT = nc.dram_tensor("cpT", [O, C], F32, kind="ExternalInput")
    vein = nc.dram_tensor("vein", [T, O], BF16, kind="ExternalInput")
    cost = nc.dram_tensor("cost", [T, NFREQ], F32, kind="ExternalInput")
    sint = nc.dram_tensor("sint", [T, NFREQ], F32, kind="ExternalInput")
    trit = nc.dram_tensor("trit", [128, 128], BF16, kind="ExternalInput")
    lam0 = nc.dram_tensor("lam0", [128, 1], F32, kind="ExternalInput")
    out_e = nc.dram_tensor("out", [T, C], F32, kind="ExternalOutput")

    timing = loop_reps > 1

    with tile.TileContext(nc) as tc:
        with (
            tc.tile_pool(name="const", bufs=1) as const,
            tc.tile_pool(name="resid", bufs=1) as resid,
            tc.tile_pool(name="xpool", bufs=3) as xpool,
            tc.tile_pool(name="vepool", bufs=2) as vepool,
            tc.tile_pool(name="sqp", bufs=2) as sqp,
            tc.tile_pool(name="small", bufs=8) as small,
            tc.tile_pool(name="qrp", bufs=2) as qrp,
            tc.tile_pool(name="qtp", bufs=2) as qtp,
            tc.tile_pool(name="ytp", bufs=2) as ytp,
            tc.tile_pool(name="ppool", bufs=4) as ppool,
            tc.tile_pool(name="opool", bufs=4) as opool,
            tc.tile_pool(name="s_ps", bufs=2, space="PSUM") as s_ps,
            tc.tile_pool(name="y_ps", bufs=2, space="PSUM") as y_ps,
            tc.tile_pool(name="g_ps", bufs=2, space="PSUM") as g_ps,
        ):
            # ---- residents (issue order = need order; small tables first,
            # qk weights before v weights; c_proj weights load in parallel
            # on the DVE DMA queue since they're needed last) ----
            cos_sb = resid.tile([128, TT, NFREQ], F32, name="cos_sb")
            nc.sync.dma_start(
                out=cos_sb[:], in_=cost.ap().rearrange("(tt p) f -> p tt f", p=128))
            sin_sb = resid.tile([128, TT, NFREQ], F32, name="sin_sb")
            nc.sync.dma_start(
                out=sin_sb[:], in_=sint.ap().rearrange("(tt p) f -> p tt f", p=128))
            tri_sb = const.tile([128, 128], BF16, name="tri_sb")
            nc.sync.dma_start(out=tri_sb[:], in_=trit.ap())
            lam0_sb = const.tile([128, 1], F32, name="lam0_sb")
            nc.sync.dma_start(out=lam0_sb[:], in_=lam0.ap())
            w_sb = resid.tile([128, CT, 3 * O], F32R, name="w_sb")
            wT_r = wT.ap().bitcast(F32R).rearrange("(ct p) o -> p ct o", p=128)
            nc.sync.dma_start(out=w_sb[:, :, 0:2 * O], in_=wT_r[:, :, 0:2 * O])
            nc.sync.dma_start(out=w_sb[:, :, 2 * O:3 * O],
                              in_=wT_r[:, :, 2 * O:3 * O])
            cp_sb = resid.tile([128, ODT, C], F32R, name="cp_sb")
            nc.scalar.dma_start(
                out=cp_sb[:], in_=cpT.ap().bitcast(F32R).rearrange("(ct p) o -> p ct o", p=128))

            ident = const.tile([128, 128], BF16, name="ident")
            make_identity(nc, ident[:])
            ones_f = const.tile([1, D], F32, name="ones_f")
            nc.vector.memset(ones_f[:], 1.0)
            ones_sb = const.tile([1, D], F32R, name="ones_sb")
            nc.vector.tensor_copy(ones_sb[:], ones_f[:])
            one128 = const.tile([128, 1], F32, name="one128")
            nc.vector.memset(one128[:], 1.0)

            kT_tiles = [resid.tile([128, ODT, NB_W], BF16, name=f"kT{i}")
                        for i in range(NB)]
            v_all = resid.tile([128, TT, HLOC, D + 1], BF16, name="v_all")
            nc.vector.tensor_copy(
                v_all[:, :, :, D:D + 1],
                one128[:, 0:1][:, None, None, :]
                .broadcast_to([128, TT, HLOC, 1]))

            xT_r = xT.ap().bitcast(F32R).rearrange("(ct p) t -> p ct t", p=128)

            # ---------- emission helpers ----------
            def make_qkv_units(i, qk_pool=None, v_pool=None,
                               pipelined=False):
                """Work units for QKV of block i (list of callables).
                qk_pool/v_pool override the PSUM pool for the projection
                accumulators (the prologue borrows the idle s/y pools so
                sub-tiles can software-pipeline); pipelined reorders the
                units so each sub-tile's DVE post-chain hides behind the
                next sub-tile's matmuls."""
                qk_pool_ = qk_pool or g_ps
                qk_tag = "s" if qk_pool is not None else "g"
                v_pool_ = v_pool or g_ps
                v_tag = "y" if v_pool is not None else "g"
                units = []
                state = {}

                def u_dma(s4):
                    def f():
                        tt = i * SUB + s4
                        x_sb = xpool.tile([128, CT, 128], F32R, tag="x", name="x_sb")
                        nc.sync.dma_start(
                            out=x_sb[:], in_=xT_r[:, :, tt * 128:(tt + 1) * 128])
                        ve_sb = vepool.tile([128, O], BF16, tag="ve", name="ve_sb")
                        nc.sync.dma_start(
                            out=ve_sb[:], in_=vein.ap()[tt * 128:(tt + 1) * 128, :])
                        state[("x", s4)] = x_sb
                        state[("ve", s4)] = ve_sb
                    return f

                def u_proj(s4, proj):
                    # proj 0=q, 1=k -> matmul chain + ACT square
                    def f():
                        x_sb = state[("x", s4)]
                        ps = qk_pool_.tile([128, O], F32, tag=qk_tag,
                                           name="ps_qk")
                        for ct in range(CT):
                            nc.tensor.matmul(
                                ps[:], lhsT=x_sb[:, ct, :],
                                rhs=w_sb[:, ct, proj * O:(proj + 1) * O],
                                start=(ct == 0), stop=(ct == CT - 1))
                        if proj == 0:
                            sqs = sqp.tile([128, 2 * O], F32, tag="sq", name="sqs")
                            state[("sqs", s4)] = sqs
                            ssq = small.tile([128, 2 * HLOC], F32, tag="sm",
                                             name="ssq")
                            state[("ssq", s4)] = ssq
                        else:
                            sqs = state[("sqs", s4)]
                            ssq = state[("ssq", s4)]
                        nc.scalar.activation(
                            sqs[:, proj * O:(proj + 1) * O], ps[:], AFT.Square)
                        nc.vector.reduce_sum(
                            ssq[:, proj * HLOC:(proj + 1) * HLOC],
                            sqs[:, proj * O:(proj + 1) * O]
                            .rearrange("p (h d) -> p h d", d=D),
                            axis=AX.X)
                        state[("ps", s4, proj)] = ps
                    return f

                def u_rstat(s4):
                    # eps + fast-inverse-sqrt (all DVE; reduces done in u_proj)
                    def f():
                        ssq = state[("ssq", s4)]
                        nc.vector.tensor_scalar(
                            out=ssq[:], in0=ssq[:], scalar1=1.0 / D, scalar2=EPS,
                            op0=OP.mult, op1=OP.add)
                        ib = small.tile([128, 2 * HLOC], I32, tag="sm", name="ib")
                        nc.vector.tensor_scalar(
                            out=ib[:], in0=ssq[:].bitcast(I32), scalar1=1,
                            scalar2=None, op0=OP.logical_shift_right)
                        nc.vector.tensor_scalar(
                            out=ib[:], in0=ib[:], scalar1=-1, scalar2=MAGIC,
                            op0=OP.mult, op1=OP.add)
                        y = ib[:].bitcast(F32)
                        t_ = small.tile([128, 2 * HLOC], F32, tag="sm", name="t")
                        for _ in range(2):
                            nc.vector.tensor_tensor(t_[:], y, y, op=OP.mult)
                            nc.vector.tensor_tensor(t_[:], t_[:], ssq[:], op=OP.mult)
                            nc.vector.tensor_scalar(
                                out=t_[:], in0=t_[:], scalar1=-0.5, scalar2=1.5,
                                op0=OP.mult, op1=OP.add)
                            nc.vector.tensor_tensor(y, y, t_[:], op=OP.mult)
                        state[("rinv", s4)] = ib
                    return f

                def u_mult(s4, proj):
                    def f():
                        ps = state[("ps", s4, proj)]
                        rinv = state[("rinv", s4)][:].bitcast(F32)
                        if proj == 0:
                            qr = qrp.tile([128, 2, HLOC, D], BF16, tag="qr", name="qr")
                            state[("qr", s4)] = qr
                        else:
                            qr = state[("qr", s4)]
                        nc.vector.tensor_tensor(
                            qr[:, proj], ps[:].rearrange("p (h d) -> p h d", d=D),
                            rinv[:, proj * HLOC:(proj + 1) * HLOC][:, :, None]
                            .broadcast_to([128, HLOC, D]),
                            op=OP.mult)
                    return f

                def u_rot(s4):
                    def f():
                        tt = i * SUB + s4
                        qr = state[("qr", s4)]
                        x1 = qr[:, :, :, 0:NFREQ]
                        x2 = qr[:, :, :, 32:32 + NFREQ]
                        cb = (cos_sb[:, tt, :][:, None, None, :]
                              .broadcast_to([128, 2, HLOC, NFREQ]))
                        sb = (sin_sb[:, tt, :][:, None, None, :]
                              .broadcast_to([128, 2, HLOC, NFREQ]))
                        rot = qrp.tile([128, 4, 2, HLOC, NFREQ], F32, tag="qr",
                                       name="rot")
                        ta, tb = rot[:, 0], rot[:, 1]
                        ua, ub = rot[:, 2], rot[:, 3]
                        # SBUF-only elementwise: run on the otherwise-idle
                        # Pool engine to keep DVE free for attention masks
                        nc.gpsimd.tensor_tensor(ta, x2, sb, op=OP.mult)
                        nc.gpsimd.tensor_tensor(tb, x2, cb, op=OP.mult)
                        nc.gpsimd.tensor_tensor(ua, x1, sb, op=OP.mult)
                        nc.gpsimd.tensor_tensor(ub, x1, cb, op=OP.mult)
                        nc.gpsimd.tensor_tensor(x1, ub, ta, op=OP.add)
                        nc.gpsimd.tensor_tensor(x2, tb, ua, op=OP.subtract)
                    return f

                def u_tp(s4, proj):
                    def f():
                        qr = state[("qr", s4)]
                        qr2 = qr[:, proj].rearrange("p h d -> p (h d)")
                        tp = g_ps.tile([128, ODT, 128], BF16, tag="g", name="tp")
                        for dt in range(ODT):
                            nc.tensor.transpose(
                                tp[:, dt, :], qr2[:, dt * 128:(dt + 1) * 128],
                                ident[:])
                        col0 = s4 * 128
                        dst = (state[("qT",)] if proj == 0 else kT_tiles[i])
                        nc.vector.tensor_copy(
                            dst[:, :, col0:col0 + 128], tp[:])
                    return f

                def u_v(s4):
                    def f():
                        x_sb = state[("x", s4)]
                        ps = v_pool_.tile([128, O], F32, tag=v_tag,
                                          name="ps_v")
                        for ct in range(CT):
                            nc.tensor.matmul(
                                ps[:], lhsT=x_sb[:, ct, :],
                                rhs=w_sb[:, ct, 2 * O:3 * O],
                                start=(ct == 0), stop=(ct == CT - 1))
                        state[("vps", s4)] = ps
                    return f

                def u_stt(s4):
                    def f():
                        tt = i * SUB + s4
                        ps = state[("vps", s4)]
                        ve_sb = state[("ve", s4)]
                        nc.vector.scalar_tensor_tensor(
                            out=v_all[:, tt, :, 0:D],
                            in0=ps[:].rearrange("p (h d) -> p h d", d=D),
                            scalar=lam0_sb[:, 0:1],
                            in1=ve_sb[:].rearrange("p (h d) -> p h d", d=D),
                            op0=OP.mult, op1=OP.add)
                    return f

                def u_qt_alloc():
                    def f():
                        state[("qT",)] = qtp.tile([128, ODT, NB_W], BF16, tag="qT", name="qT_t")
                    return f

                units.append(u_qt_alloc())
                units.append(u_dma(0))
                units.append(u_dma(1))
                if not pipelined:
                    for s4 in range(SUB):
                        if s4 + 2 < SUB:
                            units.append(u_dma(s4 + 2))
                        units.append(u_proj(s4, 0))
                        units.append(u_proj(s4, 1))
                        units.append(u_rstat(s4))
                        units.append(u_mult(s4, 0))
                        units.append(u_mult(s4, 1))
                        units.append(u_rot(s4))
                        units.append(u_v(s4))
                        units.append(u_tp(s4, 0))
                        units.append(u_stt(s4))
                        units.append(u_tp(s4, 1))
                else:
                    # software-pipelined: post-chain of s4-1 hides behind
                    # the matmul chains of s4 (needs qk on a 2-slot pool
                    # whose previous tenants release via u_mult)
                    units.append(u_dma(2))
                    units.append(u_proj(0, 0))
                    units.append(u_proj(0, 1))
                    units.append(u_rstat(0))
                    units.append(u_mult(0, 0))
                    units.append(u_mult(0, 1))
                    for s4 in range(1, SUB):
                        units.append(u_proj(s4, 0))
                        units.append(u_proj(s4, 1))
                        if s4 + 2 < SUB:
                            units.append(u_dma(s4 + 2))
                        units.append(u_rot(s4 - 1))
                        units.append(u_v(s4 - 1))
                        units.append(u_tp(s4 - 1, 0))
                        units.append(u_stt(s4 - 1))
                        units.append(u_tp(s4 - 1, 1))
                        units.append(u_rstat(s4))
                        units.append(u_mult(s4, 0))
                        units.append(u_mult(s4, 1))
                    s4 = SUB - 1
                    units.append(u_rot(s4))
                    units.append(u_v(s4))
                    units.append(u_tp(s4, 0))
                    units.append(u_stt(s4))
                    units.append(u_tp(s4, 1))
                return units, state

            def make_cproj_units(i, yt_sb, pool=None, ptag="y"):
                """c_proj partials for block i's yt; 8 units of (s4, ob)."""
                units = []

                def u_cp(s4, ob):
                    def f():
                        r0 = (i * SUB + s4) * 128
                        cps = (pool or y_ps).tile([128, OBW], F32, tag=ptag,
                                                  name="cps")
                        for ct in range(ODT):
                            nc.tensor.matmul(
                                cps[:],
                                lhsT=yt_sb[:, ct, s4 * 128:(s4 + 1) * 128],
                                rhs=cp_sb[:, ct, ob * OBW:(ob + 1) * OBW],
                                start=(ct == 0), stop=(ct == ODT - 1))
                        o_sb = opool.tile([128, OBW], F32, tag="o", name="o_sb")
                        nc.scalar.copy(o_sb[:], cps[:])
                        nc.sync.dma_start(
                            out=out_e.ap()[r0:r0 + 128,
                                           ob * OBW:(ob + 1) * OBW],
                            in_=o_sb[:])
                    return f

                for s4 in range(SUB):
                    for ob in range(OB):
                        units.append(u_cp(s4, ob))
                return units

            pending = {"tail": None}

            def emit_attention(i, qT_sb, yt_sb, cproj_units, bg_units):
                """Emit attention block i with cross-pair software
                pipelining: each pair's last two PVs + normalize are
                deferred into a tail that fires after the NEXT pair's
                first two S tiles (hiding the normalize chain), pulling
                two cproj units right after each normalize (they reuse
                the freed y banks).  bg_units (qkv of i+1) weave evenly
                between j-steps."""
                njt = (i + 1) * SUB
                ticks = max(NPAIR * (njt - 1), 1)
                ratio = len(bg_units) / ticks
                wove = {"acc": 0.0, "bi": 0}
                cp = {"i": 0}

                def weave_tick():
                    wove["acc"] += ratio
                    while wove["bi"] < len(bg_units) and wove["bi"] < int(wove["acc"]):
                        bg_units[wove["bi"]]()
                        wove["bi"] += 1

                def pull_cproj(n=2):
                    while n > 0 and cp["i"] < len(cproj_units):
                        cproj_units[cp["i"]]()
                        cp["i"] += 1
                        n -= 1

                for hp in range(NPAIR):
                    pair = (2 * hp, 2 * hp + 1)
                    ctx = {"ypss": None}
                    s_hist = []  # (j, w0, p_tile)

                    def emit_S(j, s_hist=s_hist, pair=pair):
                        m = j - i * SUB
                        w0 = 128 * m if m >= 0 else 0
                        st = s_ps.tile([128, 2, NB_W], F32, tag="s", name="st")
                        for hh, h in enumerate(pair):
                            po = (h % 2) * D
                            dt = h // 2
                            nc.tensor.matmul(
                                st[:, hh, w0:NB_W],
                                lhsT=kT_tiles[j // SUB][po:po + D, dt,
                                                        (j % SUB) * 128:
                                                        (j % SUB) * 128 + 128],
                                rhs=qT_sb[po:po + D, dt, w0:NB_W],
                                start=True, stop=True)
                        pt = ppool.tile([128, 2, NB_W], BF16, tag="p", name="pt")
                        if w0 == 0:
                            nc.scalar.activation(
                                pt[:].rearrange("p a b -> p (a b)"),
                                st[:].rearrange("p a b -> p (a b)"),
                                AFT.Exp, bias=0.0, scale=ATTN_SCALE)
                        else:
                            # contiguous per-head APs: strided multi-dim
                            # APs measurably slow ACT on hardware
                            for hh in range(2):
                                nc.scalar.activation(
                                    pt[:, hh, w0:NB_W], st[:, hh, w0:NB_W],
                                    AFT.Exp, bias=0.0, scale=ATTN_SCALE)
                        if m >= 0:
                            for hh in range(2):
                                nc.vector.tensor_tensor(
                                    pt[:, hh, w0:w0 + 128],
                                    pt[:, hh, w0:w0 + 128],
                                    tri_sb[:], op=OP.mult)
                        s_hist.append((j, w0, pt))

                    def emit_PV(idx, s_hist=s_hist, ctx=ctx, pair=pair, njt=njt):
                        j, w0, pt = s_hist[idx]
                        for hh, h in enumerate(pair):
                            nc.tensor.matmul(
                                ctx["ypss"][hh][:, w0:NB_W],
                                lhsT=v_all[:, j, h, :],
                                rhs=pt[:, hh, w0:NB_W],
                                start=(j == 0), stop=(j == njt - 1))

                    def make_tail(emit_PV=emit_PV, ctx=ctx, pair=pair,
                                  njt=njt, yt_dst=yt_sb):
                        def tail(pull):
                            emit_PV(njt - 2)
                            emit_PV(njt - 1)
                            for hh, h in enumerate(pair):
                                po = (h % 2) * D
                                dt = h // 2
                                yps = ctx["ypss"][hh]
                                rec = opool.tile([1, NB_W], F32R, tag="o",
                                                 name="rec")
                                with nc.allow_low_precision(
                                        reason="f32r recip for broadcast"):
                                    nc.vector.reciprocal(rec[:],
                                                         yps[D:D + 1, :])
                                bc = s_ps.tile([D, NB_W], F32, tag="s",
                                               name="bc")
                                nc.tensor.matmul(
                                    bc[:], lhsT=ones_sb[:], rhs=rec[:],
                                    start=True, stop=True)
                                bcs = opool.tile([D, NB_W], F32, tag="o",
                                                 name="bcs")
                                nc.vector.tensor_copy(bcs[:], bc[:])
                                nc.vector.tensor_tensor(
                                    yt_dst[po:po + D, dt, :], yps[0:D, :],
                                    bcs[:], op=OP.mult)
                            pull(2)
                        return tail

                    emit_S(0)
                    emit_S(1)
                    if pending["tail"] is not None:
                        pending["tail"](pull_cproj)
                    ctx["ypss"] = [y_ps.tile([D + 1, NB_W], F32, tag="y",
                                             name=f"yps{hh}")
                                   for hh in range(2)]
                    for j in range(2, njt):
                        emit_S(j)
                        weave_tick()
                        emit_PV(j - 2)
                    pending["tail"] = make_tail()
                # flush leftover background units
                while wove["bi"] < len(bg_units):
                    bg_units[wove["bi"]]()
                    wove["bi"] += 1
                return pull_cproj

            # ---------- main pipeline ----------
            import contextlib
            loop_cm = (tc.For_i(0, loop_reps, 1) if timing
                       else contextlib.nullcontext())
            with loop_cm:
                qkv_units, qkv_state = make_qkv_units(0)
                for u in qkv_units:
                    u()
                qT_cur = qkv_state[("qT",)]
                yt_prev = None
                for i in range(NB):
                    yt_sb = ytp.tile([128, ODT, NB_W], F32R, tag="yt", name="yt_sb")
                    if i + 1 < NB:
                        nxt_units, nxt_state = make_qkv_units(i + 1)
                        cp_units = (make_cproj_units(i - 1, yt_prev)
                                    if i > 0 else [])
                    else:
                        # last block: no next qkv; weave c_proj(i-1) through
                        # the j-loop instead (qkv's g_ps banks are free now)
                        nxt_units = make_cproj_units(i - 1, yt_prev,
                                                     pool=g_ps, ptag="g")
                        nxt_state = None
                        cp_units = []
                    emit_attention(i, qT_cur, yt_sb, cp_units, nxt_units)
                    if nxt_state is not None:
                        qT_cur = nxt_state[("qT",)]
                    yt_prev = yt_sb
                # epilogue: final pair tail + c_proj of the last block
                epi_units = make_cproj_units(NB - 1, yt_prev)
                epi = {"i": 0}

                def pull_epi(n=2):
                    while n > 0 and epi["i"] < len(epi_units):
                        epi_units[epi["i"]]()
                        epi["i"] += 1
                        n -= 1

                pending["tail"](pull_epi)
                pending["tail"] = None
                while epi["i"] < len(epi_units):
                    epi_units[epi["i"]]()
                    epi["i"] += 1

    nc.compile()
    return nc


def make_tables(T):
    angular = (np.float32(1.0 / 1024.0)
               ** np.linspace(0.0, 1.0, NFREQ, dtype=np.float32))
    t = np.arange(T, dtype=np.float32)
    theta = t[:, None] * angular[None, :]
    return np.cos(theta).astype(np.float32), np.sin(theta).astype(np.float32)


def make_tri(dtype):
    # tri[r, c] = 1 where c >= r (keep), else 0
    return np.triu(np.ones((128, 128), np.float32)).astype(dtype)


def prep_core_inputs(x, ve, qkv_w, lambdas, c_proj_w, core, n_groups=2):
    import ml_dtypes
    bf16 = ml_dtypes.bfloat16
    T = x.shape[1]
    C = x.shape[2]
    O = C // n_groups
    b, g = core // n_groups, core % n_groups
    cols = slice(g * O, (g + 1) * O)
    xT = np.ascontiguousarray(x[b].T)
    wTm = np.concatenate(
        [np.ascontiguousarray(qkv_w[p, cols, :].T) for p in range(3)], axis=1)
    cpT = np.ascontiguousarray(c_proj_w[:, cols].T)
    cos_t, sin_t = make_tables(T)
    return {
        "xT": xT,
        "wT": np.ascontiguousarray(wTm),
        "cpT": cpT,
        "vein": np.ascontiguousarray(
            (np.float32(lambdas[1]) * ve[b, :, cols]).astype(bf16)),
        "cost": cos_t,
        "sint": sin_t,
        "trit": make_tri(bf16),
        "lam0": np.full((128, 1), lambdas[0], np.float32),
    }


_NC_CACHE = {}


def get_nc(T, C, O):
    key = (T, C, O)
    if key not in _NC_CACHE:
        _NC_CACHE[key] = build_nc(T, C, O)
    return _NC_CACHE[key]


def kernel(x, ve, qkv_w, lambdas, c_proj_w):
    x = np.asarray(x, np.float32)
    ve = np.asarray(ve, np.float32)
    qkv_w = np.asarray(qkv_w, np.float32)
    lambdas = np.asarray(lambdas, np.float32)
    c_proj_w = np.asarray(c_proj_w, np.float32)
    B, T, C = x.shape
    O = C // 2
    nc = get_nc(T, C, O)
    in_maps = [prep_core_inputs(x, ve, qkv_w, lambdas, c_proj_w, c)
               for c in range(N_CORES)]
    res = run_bass_kernel_spmd(nc, in_maps, list(range(N_CORES)))
    out = np.stack([res.results[2 * b]["out"] + res.results[2 * b + 1]["out"]
                    for b in range(B)])
    return out


def measure_hw_time_ns(inputs, r1=8, r2=2008, runs=3):
    """Slope-timing: in-NEFF For_i repetition, min-wall over runs."""
    import time as _time
    x = np.asarray(inputs["x"], np.float32)
    B, T, C = x.shape
    O = C // 2
    in_maps = [prep_core_inputs(x, np.asarray(inputs["ve"], np.float32),
                                np.asarray(inputs["qkv_w"], np.float32),
                                np.asarray(inputs["lambdas"], np.float32),
                                np.asarray(inputs["c_proj_w"], np.float32), c)
               for c in range(N_CORES)]
    times = {}
    for reps in (r1, r2):
        nc = build_nc(T, C, O, loop_reps=reps)
        best = float("inf")
        for _ in range(runs):
            t0 = _time.time()
            run_bass_kernel_spmd(nc, in_maps, list(range(N_CORES)))
            best = min(best, _time.time() - t0)
        times[reps] = best
    return (times[r2] - times[r1]) / (r2 - r1) * 1e9
